# revision 98
# baseline (speedup 1.0000x reference)
"""Trainium2 Bass kernel for nn_BaseMPNN (GNN message passing), 8-core SPMD.

Strategy (edge-parallel, destination-sorted, balanced windows):
- Nodes are LPT-permuted within each core (by in-degree) so every 128-node
  window has a near-equal edge count; edges are routed to the core owning
  their destination and packed into per-window slots with fixed capacity
  CPW*128.
- The replicated h table is split into two DRAM tables A/B (windows 0..WA-1
  vs the rest, each + one all-zero pad window) so BOTH fit int16 dma_gather
  indexing; each window's slots are segregated by source table with fixed
  sub-capacities LCH/HCH chunks, so the two h[row] gathers fetch only their
  half's rows (transpose mode lands feature-major directly).
- h[col] is never gathered: since edges are destination-sorted, the wef1
  term of the edge model is hw2 = h_win @ wef1 (one matmul per window,
  hoisted to layer start) contracted against a host-precomputed raw one-hot
  transpose streamed from DRAM. The scatter-mean is per-chunk matmuls
  against the host-precomputed scaled one-hot t1w[e,n] = (colrel==n)/deg.
- Everything runs in bf16 with f32 PSUM. mlp1 bias enters via a DVE PSUM
  pre-load; node-major h windows for the table come from PE transposes of
  the feature-major h (no second mlp2 orientation).
- BN (training stats) is folded into the next layer's weights; sums come
  from full-row DVE reduces + Square-activation accumulations, with the
  constant pad-column contribution subtracted analytically (exact, bf16
  rounding mirrored on device).
- Cross-core: per layer, table A's AllGather issues after the last gather
  of the sweep (hidden under tail compute; collectives block the GpSimd
  queue), then the stats AllReduce, then table B's AllGather; the next
  layer's first two A-gathers are issued between A and B so their
  descriptor generation overlaps the B transfer.

Assumes the problem instance has all-zero biases (true for seed-0
setup_inputs; asserted in add_weight_params) so padded inputs stay exactly
zero at the encoder.
"""

import math
from contextlib import ExitStack
from dataclasses import dataclass

import numpy as np

import concourse.bacc as bacc
import concourse.bass as bass
import concourse.tile as tile
from concourse import mybir
from concourse.masks import make_identity

F32 = mybir.dt.float32
BF16 = mybir.dt.bfloat16
I16 = mybir.dt.int16
P = 128
EPS = 1e-5
SPLIT = 32768


@dataclass(frozen=True)
class Cfg:
    NC: int = 8        # cores
    H: int = 128       # hidden (must be 128)
    F: int = 16        # input features
    L: int = 3         # meta layers
    NW: int = 49       # 128-node windows per core
    CPW: int = 5       # 128-edge chunks per window (capacity)
    GW: int = 7        # windows per gather instruction
    N_real: int = 50000
    E_real: int = 200000

    LCH: int = 3       # lo-half (table A) chunks per window
    HCH: int = 2       # hi-half (table B) chunks per window
    WA: int = 30       # windows whose nodes live in table A (rest in B)

    @property
    def WB(self):
        return self.NW - self.WA

    @property
    def SEGA(self):  # table-A rows per core (incl. zero pad window)
        return (self.WA + 1) * P

    @property
    def SEGB(self):
        return (self.WB + 1) * P

    @property
    def NTA(self):   # global table-A rows
        return self.NC * self.SEGA

    @property
    def NTB(self):
        return self.NC * self.SEGB

    @property
    def ZA(self):    # reserved zero row in table A (core0 pad window)
        return self.WA * P

    @property
    def ZB(self):
        return self.WB * P

    @property
    def LOW(self):   # lo slots per window
        return self.LCH * P

    @property
    def HIW(self):   # hi slots per window
        return self.HCH * P

    @property
    def NIRL(self):  # lo gather rows per group
        return self.GW * self.LOW

    @property
    def NIRH(self):  # hi gather rows per group
        return self.GW * self.HIW

    @property
    def NPC(self):   # compute nodes per core
        return self.NW * P

    @property
    def NPCT(self):  # node-table rows per core (incl. zero pad window)
        return (self.NW + 1) * P

    @property
    def NPADT(self):  # global node-table rows
        return self.NC * self.NPCT

    @property
    def ECAP(self):  # edge slots per core
        return self.NW * self.CPW * P

    @property
    def CHUNKS(self):
        return self.NW * self.CPW

    @property
    def NG(self):    # gather groups
        return math.ceil(self.NW / self.GW)

    @property
    def NIR(self):   # gather rows per group instruction
        return self.GW * self.CPW * P

    @property
    def ZLO(self):   # reserved zero row in the low half (core0 pad window)
        return self.NW * P

    @property
    def use_hi(self):
        return self.NPADT > SPLIT

    @property
    def ZHI(self):   # reserved zero row in the high half (relative)
        return self.NPADT - 1 - SPLIT if self.use_hi else 0


def _wrap16(flat):
    """int16 flat index list -> [128, n/16] wrap-16, replicated x8 groups."""
    n = len(flat)
    assert n % 16 == 0
    w = flat.reshape(n // 16, 16).T
    return np.ascontiguousarray(np.tile(w, (8, 1)))


def prep(cfg: Cfg, x, edge_index, edge_attr):
    """Host-side preprocessing -> per-core input maps (index metadata only).

    Nodes are permuted within each core (greedy LPT on in-degree) so window
    edge counts are balanced; each window's slots are split lo/hi by source
    table row (< SPLIT or not) with fixed sub-capacities LOW/HIW, so the two
    h[row] gathers fetch only their half's rows.
    """
    import heapq

    import ml_dtypes

    x = np.asarray(x, np.float32)
    ei = np.asarray(edge_index, np.int64)
    ea = np.asarray(edge_attr, np.float32)
    row, col = ei[0], ei[1]
    NPC, NW, CPW, ECAP = cfg.NPC, cfg.NW, cfg.CPW, cfg.ECAP
    LOW, HIW = cfg.LOW, cfg.HIW
    NALL = cfg.NC * NPC

    deg = np.bincount(col, minlength=NALL).astype(np.int64)
    rdeg_all = 1.0 / np.maximum(deg, 1.0).astype(np.float32)

    x_pad = np.zeros((NALL, cfg.F), np.float32)
    x_pad[: cfg.N_real] = x

    # per-core LPT window balancing -> node position permutation
    pos_of = np.empty(NALL, np.int64)   # node id -> position within core
    order_all = np.empty(NALL, np.int64)  # core*NPC+pos -> node id
    for c in range(cfg.NC):
        nodes = np.arange(c * NPC, (c + 1) * NPC)
        dg = deg[nodes]
        order = np.argsort(-dg, kind="stable")
        heap = [(0, w) for w in range(NW)]
        heapq.heapify(heap)
        loads = np.zeros(NW, np.int64)
        counts = np.zeros(NW, np.int64)
        win_of = np.empty(NPC, np.int64)
        for idx in order:
            while True:
                load, w = heapq.heappop(heap)
                if counts[w] < P and loads[w] == load:
                    break
            win_of[idx] = w
            loads[w] += dg[idx]
            counts[w] += 1
            if counts[w] < P:
                heapq.heappush(heap, (int(loads[w]), w))
        assert np.all(counts == P)
        ordered = np.argsort(win_of, kind="stable")
        pos = np.empty(NPC, np.int64)
        pos[ordered] = np.arange(NPC)
        pos_of[nodes] = pos
        order_all[c * NPC : (c + 1) * NPC] = nodes[ordered]

    core_idx = np.arange(NALL) // NPC
    is_a_node = pos_of < cfg.WA * P
    trow_a = core_idx * cfg.SEGA + pos_of
    trow_b = core_idx * cfg.SEGB + (pos_of - cfg.WA * P)

    core_of = col // NPC
    maps = []
    for c in range(cfg.NC):
        sel = np.nonzero(core_of == c)[0]
        order = np.argsort(pos_of[col[sel]], kind="stable")
        sel = sel[order]
        ecol = col[sel]
        epos = pos_of[ecol]
        erow = row[sel]
        eatt = ea[sel]
        win = epos // P
        rt_a = trow_a[erow]
        rt_b = trow_b[erow]
        is_lo = is_a_node[erow]

        r_lo = np.full(NW * LOW, cfg.ZA, np.int64)
        r_hi = np.full(NW * HIW, cfg.ZB, np.int64)
        colrel_i = np.zeros(ECAP, np.int64)
        redge = np.zeros(ECAP, np.float32)
        real = np.zeros(ECAP, bool)
        ea_slots = np.zeros((ECAP, cfg.F), np.float32)
        for w in range(NW):
            wl = np.nonzero((win == w) & is_lo)[0]
            wh = np.nonzero((win == w) & ~is_lo)[0]
            nl, nh = len(wl), len(wh)
            assert nl <= LOW, f"lo overflow core {c} win {w}: {nl}"
            assert nh <= HIW, f"hi overflow core {c} win {w}: {nh}"
            s = w * CPW * P
            r_lo[w * LOW : w * LOW + nl] = rt_a[wl]
            r_hi[w * HIW : w * HIW + nh] = rt_b[wh]
            for (slo, wsel, cnt) in ((s, wl, nl), (s + LOW, wh, nh)):
                colrel_i[slo : slo + cnt] = epos[wsel] % P
                redge[slo : slo + cnt] = rdeg_all[ecol[wsel]]
                real[slo : slo + cnt] = True
                ea_slots[slo : slo + cnt] = eatt[wsel]

        onehot = np.zeros((ECAP, P), np.float32)
        ridx = np.nonzero(real)[0]
        onehot[ridx, colrel_i[ridx]] = 1.0
        t1w_full = onehot * redge[:, None]
        t1w_t = (
            t1w_full.reshape(cfg.CHUNKS, P, P)
            .transpose(1, 0, 2)
            .reshape(P, cfg.CHUNKS * P)
        )
        oht_t = (
            onehot.reshape(cfg.CHUNKS, P, P)
            .transpose(2, 0, 1)
            .reshape(P, cfg.CHUNKS * P)
        )

        NG = cfg.NG

        def wrap_groups(a, per_win, fill):
            nir = cfg.GW * per_win
            g = np.full(NG * nir, fill, np.int64)
            g[: NW * per_win] = a
            assert g.max() < SPLIT and g.min() >= 0
            cols = [
                _wrap16(g[gi * nir : (gi + 1) * nir].astype(np.int16))
                for gi in range(NG)
            ]
            return np.concatenate(cols, axis=1)

        ilo = wrap_groups(r_lo, LOW, cfg.ZA)
        ihi = wrap_groups(r_hi, HIW, cfg.ZB)

        gid = c * NPC + np.arange(NPC)
        npad_nodes = int(np.sum(gid >= cfg.N_real))

        maps.append(
            {
                "npads": np.full((P, 1), ECAP - len(sel), np.float32),
                "npadn": np.full((P, 1), npad_nodes, np.float32),
                "xT": np.ascontiguousarray(
                    x_pad[order_all[c * NPC : (c + 1) * NPC]].T
                ),
                "eaT": np.ascontiguousarray(ea_slots.T),
                "ilo": ilo,
                "ihi": ihi,
                "t1w": np.ascontiguousarray(t1w_t.astype(ml_dtypes.bfloat16)),
                "oht": np.ascontiguousarray(oht_t.astype(ml_dtypes.bfloat16)),
            }
        )
    return maps


def add_weight_params(cfg: Cfg, maps, w):
    """Append (replicated) weight arrays to each core's input map."""
    H, L = cfg.H, cfg.L

    def col(a):
        return np.asarray(a, np.float32).reshape(H, 1)

    shared = {
        "enc_node_w": np.asarray(w["enc_node_w"], np.float32),
        "enc_edge_w": np.asarray(w["enc_edge_w"], np.float32),
        "enc_node_b_col": col(w["enc_node_b"]),
        "enc_edge_b_col": col(w["enc_edge_b"]),
        "edge_w": np.asarray(w["edge_w"], np.float32),
        "edge_b_col": np.asarray(w["edge_b"], np.float32).reshape(L, H, 1),
        "n1_w": np.asarray(w["n1_w"], np.float32),
        "n1_b_col": np.asarray(w["n1_b"], np.float32).reshape(L, H, 1),
        "n2_w": np.asarray(w["n2_w"], np.float32),
        "n2_b_col": np.asarray(w["n2_b"], np.float32).reshape(L, H, 1),
        "bn_node_g": col(w["bn_node_g"]),
        "bn_node_b": col(w["bn_node_b"]),
        "bn_edge_g": col(w["bn_edge_g"]),
        "bn_edge_b": col(w["bn_edge_b"]),
        "reg_w": np.asarray(w["reg_w"], np.float32).reshape(2 * H, 1),
        "reg_b": np.asarray(w["reg_b"], np.float32).reshape(1, 1),
    }
    for k in ["enc_node_b", "enc_edge_b", "edge_b", "n1_b", "n2_b"]:
        assert np.all(np.asarray(w[k]) == 0.0), f"nonzero bias {k} unsupported"
    for m in maps:
        m.update(shared)
    return maps


def build(cfg: Cfg, reps: int = 1):
    """Build the SPMD Bass program. Returns nc."""
    H, F, L, NW, CPW, GW = cfg.H, cfg.F, cfg.L, cfg.NW, cfg.CPW, cfg.GW
    NPC, NPCT, NPADT, ECAP = cfg.NPC, cfg.NPCT, cfg.NPADT, cfg.ECAP
    NG = cfg.NG
    LOW, HIW, NIRL, NIRH = cfg.LOW, cfg.HIW, cfg.NIRL, cfg.NIRH
    SWL, SWH = NIRL // 16, NIRH // 16
    WSL = CPW * P
    PSE_HI = 512  # psum column offset of the hi region (bank-aligned)
    GA_GROUP = (cfg.WA + GW - 1) // GW - 1  # group completing table A's windows
    inv_n = 1.0 / cfg.N_real
    inv_e = 1.0 / cfg.E_real

    nc = bacc.Bacc(
        "TRN2", target_bir_lowering=False, debug=False, num_devices=cfg.NC
    )

    def param(name, shape, dt=F32):
        return nc.declare_dram_parameter(name, list(shape), dt, isOutput=False).ap()

    xT = param("xT", [F, NPC])
    eaT = param("eaT", [F, ECAP])
    ilo_p = param("ilo", [P, NG * SWL], I16)
    ihi_p = param("ihi", [P, NG * SWH], I16)
    t1w_p = param("t1w", [P, cfg.CHUNKS * P], BF16)
    oht_p = param("oht", [P, cfg.CHUNKS * P], BF16)
    npads_p = param("npads", [P, 1])
    npadn_p = param("npadn", [P, 1])
    enc_node_w = param("enc_node_w", [F, H])
    enc_edge_w = param("enc_edge_w", [F, H])
    enc_node_b_col = param("enc_node_b_col", [H, 1])
    enc_edge_b_col = param("enc_edge_b_col", [H, 1])
    edge_w_p = param("edge_w", [L, 3 * H, H])
    edge_b_col_p = param("edge_b_col", [L, H, 1])
    n1_w_p = param("n1_w", [L, 2 * H, H])
    n1_b_col_p = param("n1_b_col", [L, H, 1])
    n2_w_p = param("n2_w", [L, 2 * H, H])
    n2_b_col_p = param("n2_b_col", [L, H, 1])
    bn_node_g = param("bn_node_g", [H, 1])
    bn_node_b = param("bn_node_b", [H, 1])
    bn_edge_g = param("bn_edge_g", [H, 1])
    bn_edge_b = param("bn_edge_b", [H, 1])
    reg_w_p = param("reg_w", [2 * H, 1])
    reg_b_p = param("reg_b", [1, 1])
    out_p = nc.declare_dram_parameter("out", [1, 1], F32, isOutput=True).ap()

    eT_d = [nc.dram_tensor(f"eT_{i}", [P, ECAP], BF16).ap() for i in range(2)]
    hsegA = [
        nc.dram_tensor(f"hsegA_{i}", [cfg.SEGA, H], BF16).ap() for i in range(L)
    ]
    hsegB = [
        nc.dram_tensor(f"hsegB_{i}", [cfg.SEGB, H], BF16).ap() for i in range(L)
    ]
    htabA = [
        nc.dram_tensor(f"htabA_{i}", [cfg.NTA, H], BF16, addr_space="Shared").ap()
        for i in range(L)
    ]
    htabB = [
        nc.dram_tensor(f"htabB_{i}", [cfg.NTB, H], BF16, addr_space="Shared").ap()
        for i in range(L)
    ]
    ar_in = [nc.dram_tensor(f"ar_in_{i}", [H, 4], F32).ap() for i in range(L)]
    ar_out = [
        nc.dram_tensor(f"ar_out_{i}", [H, 4], F32, addr_space="Shared").ap()
        for i in range(L)
    ]
    rg = [list(range(cfg.NC))]

    AluOp = mybir.AluOpType
    Act = mybir.ActivationFunctionType

    with tile.TileContext(nc) as tc, ExitStack() as ctx:
        singles = ctx.enter_context(tc.tile_pool(name="singles", bufs=1))
        wpool = ctx.enter_context(tc.tile_pool(name="wpool", bufs=3))
        cpool = ctx.enter_context(tc.tile_pool(name="cpool", bufs=3))
        gpool = ctx.enter_context(tc.tile_pool(name="gpool", bufs=4))
        stpool = ctx.enter_context(tc.tile_pool(name="stpool", bufs=3))
        spool = ctx.enter_context(tc.tile_pool(name="spool", bufs=1))
        ps_e = ctx.enter_context(tc.tile_pool(name="ps_e", bufs=2, space="PSUM"))
        ps_m = ctx.enter_context(tc.tile_pool(name="ps_m", bufs=2, space="PSUM"))
        ps_agg = ctx.enter_context(
            tc.tile_pool(name="ps_agg", bufs=2, space="PSUM")
        )

        ones_row = singles.tile([1, P], F32)
        nc.vector.memset(ones_row[:], 1.0)
        ident1 = singles.tile([1, 1], F32)
        nc.vector.memset(ident1[:], 1.0)
        ident_f = singles.tile([P, P], F32)
        make_identity(nc, ident_f[:])
        ident_bf = singles.tile([P, P], BF16)
        nc.vector.tensor_copy(out=ident_bf[:], in_=ident_f[:])
        eps_sb = singles.tile([P, 1], F32)
        nc.vector.memset(eps_sb[:], EPS)
        zero_bf = singles.tile([P, P], BF16)
        nc.vector.memset(zero_bf[:], 0.0)

        def load(name_, shape, src, dt=F32, pool=singles):
            t = pool.tile(shape, dt, tag=name_, name=name_)
            nc.sync.dma_start(out=t[:], in_=src)
            return t

        ilo_sb = load("ilo_sb", [P, NG * SWL], ilo_p[:, :], I16)
        ihi_sb = load("ihi_sb", [P, NG * SWH], ihi_p[:, :], I16)
        npads_sb = load("npads_sb", [P, 1], npads_p[:, :])
        npadn_sb = load("npadn_sb", [P, 1], npadn_p[:, :])
        encn_w = load("encn_w", [F, H], enc_node_w[:, :])
        ence_w = load("ence_w", [F, H], enc_edge_w[:, :])
        encn_b = load("encn_b", [H, 1], enc_node_b_col[:, :])
        ence_b = load("ence_b", [H, 1], enc_edge_b_col[:, :])
        w_e = [
            [load(f"w_e_{i}_{k}", [P, H], edge_w_p[i, k * P : (k + 1) * P, :])
             for k in range(3)]
            for i in range(L)
        ]
        w_n1 = [
            [load(f"w_n1_{i}_{k}", [P, H], n1_w_p[i, k * P : (k + 1) * P, :])
             for k in range(2)]
            for i in range(L)
        ]
        w_n2 = [
            [load(f"w_n2_{i}_{k}", [P, H], n2_w_p[i, k * P : (k + 1) * P, :])
             for k in range(2)]
            for i in range(L)
        ]
        be_col = [load(f"be_{i}", [H, 1], edge_b_col_p[i, :, :]) for i in range(L)]
        b1_col = [load(f"b1_{i}", [H, 1], n1_b_col_p[i, :, :]) for i in range(L)]
        b2_col = [load(f"b2_{i}", [H, 1], n2_b_col_p[i, :, :]) for i in range(L)]
        bng = load("bng", [H, 1], bn_node_g[:, :])
        bnb = load("bnb", [H, 1], bn_node_b[:, :])
        beg = load("beg", [H, 1], bn_edge_g[:, :])
        beb = load("beb", [H, 1], bn_edge_b[:, :])
        regw_h = load("regw_h", [P, 1], reg_w_p[0:P, :])
        regw_e = load("regw_e", [P, 1], reg_w_p[P : 2 * P, :])
        regb_sb = load("regb_sb", [1, 1], reg_b_p[:, :])

        # bf16 copies of raw n1/n2 chunk-1 weights (never folded)
        n1b1 = []
        n2b1 = []
        for i in range(L):
            t = singles.tile([P, H], BF16, tag=f"n1b1_{i}", name=f"n1b1_{i}")
            nc.vector.tensor_copy(out=t[:], in_=w_n1[i][1][:])
            n1b1.append(t)
            t2 = singles.tile([P, H], BF16, tag=f"n2b1_{i}", name=f"n2b1_{i}")
            nc.vector.tensor_copy(out=t2[:], in_=w_n2[i][1][:])
            n2b1.append(t2)

        hbf = [
            singles.tile([P, NPC], BF16, tag=f"hbf_{s}", name=f"hbf_{s}")
            for s in range(2)
        ]

        def copy_dve(dst, src):
            nc.vector.tensor_copy(out=dst, in_=src)

        def issue_glo_for(li, g):
            t = gpool.tile([P, 1, NIRL], BF16, tag="glo", name="glo")
            nc.gpsimd.dma_gather(
                out_ap=t[:], in_ap=htabA[li][:, :],
                idxs_ap=ilo_sb[:, g * SWL : (g + 1) * SWL],
                num_idxs=NIRL, num_idxs_reg=NIRL, elem_size=H,
                transpose=True, single_packet=False,
            )
            return t

        glo_pref = {}

        for _rep in range(reps):
            # ================= encoder =================
            for li in range(L):
                nc.sync.dma_start(
                    out=hsegA[li][cfg.ZA : cfg.SEGA, :], in_=zero_bf[:]
                )
                nc.sync.dma_start(
                    out=hsegB[li][cfg.ZB : cfg.SEGB, :], in_=zero_bf[:]
                )
            nsl = [(s, min(s + 512, NPC)) for s in range(0, NPC, 512)]
            for (s0, s1) in nsl:
                xsl = wpool.tile([F, 512], F32, tag="xsl", name="xsl")
                nc.sync.dma_start(out=xsl[:, : s1 - s0], in_=xT[:, s0:s1])
                pse = ps_e.tile([P, 1024], F32, tag="pse_w", name="pse")
                nc.tensor.matmul(
                    out=pse[:, : s1 - s0], lhsT=encn_w[:], rhs=xsl[:, : s1 - s0],
                    start=True, stop=True,
                )
                nc.scalar.activation(
                    out=hbf[0][:, s0:s1], in_=pse[:, : s1 - s0], func=Act.Relu,
                    bias=encn_b[:, 0:1],
                )
            for w in range(NW):
                ws = w * P
                pst2 = ps_m.tile([P, P], BF16, tag="psm", name="pst2")
                nc.tensor.transpose(
                    out=pst2[:], in_=hbf[0][:, ws : ws + P], identity=ident_bf[:]
                )
                hwin = cpool.tile([P, P], BF16, tag="hwin", name="hwin")
                nc.scalar.activation(out=hwin[:], in_=pst2[:], func=Act.Copy)
                if w < cfg.WA:
                    nc.sync.dma_start(
                        out=hsegA[0][ws : ws + P, :], in_=hwin[:]
                    )
                else:
                    wb = (w - cfg.WA) * P
                    nc.sync.dma_start(
                        out=hsegB[0][wb : wb + P, :], in_=hwin[:]
                    )
                if w == cfg.WA - 1:
                    nc.gpsimd.collective_compute(
                        "AllGather", AluOp.bypass, replica_groups=rg,
                        ins=[hsegA[0][:, :]], outs=[htabA[0][:, :]],
                    )
                    for gg in range(min(2, NG)):
                        glo_pref[gg] = issue_glo_for(0, gg)
            nc.gpsimd.collective_compute(
                "AllGather", AluOp.bypass, replica_groups=rg,
                ins=[hsegB[0][:, :]], outs=[htabB[0][:, :]],
            )
            for w in range(NW):
                es = w * WSL
                ea_sb = wpool.tile([F, WSL], F32, tag="ea_sb", name="ea_sb")
                nc.sync.dma_start(out=ea_sb[:], in_=eaT[:, es : es + WSL])
                pse2 = ps_e.tile([P, 1024], F32, tag="pse_w", name="pse2")
                for (s0, s1) in [(0, 512), (512, WSL)] if WSL > 512 else [(0, WSL)]:
                    nc.tensor.matmul(
                        out=pse2[:, s0:s1], lhsT=ence_w[:], rhs=ea_sb[:, s0:s1],
                        start=True, stop=True,
                    )
                enT = wpool.tile([P, WSL], BF16, tag="enT", name="enT")
                nc.scalar.activation(
                    out=enT[:], in_=pse2[:, :WSL], func=Act.Relu, bias=ence_b[:, 0:1]
                )
                nc.sync.dma_start(out=eT_d[0][:, es : es + WSL], in_=enT[:])

            # ================= layers =================
            epad_bf = spool.tile([P, 1], BF16, tag="epad_bf_a", name="epad_bf")
            nc.vector.memset(epad_bf[:], 0.0)
            hpad = spool.tile([P, 1], BF16, tag="hpad_a", name="hpad")
            nc.vector.memset(hpad[:], 0.0)
            s_h = t_h = s_e = t_e = None
            for i in range(L):
                last = i == L - 1
                h_cur, h_nxt = hbf[i % 2], hbf[(i + 1) % 2]
                eT_cur, eT_nxt = eT_d[i % 2], eT_d[(i + 1) % 2]

                # ---- fold BN into this layer's weights ----
                if i == 0:
                    wef = []
                    for k in range(3):
                        t = spool.tile([P, H], BF16, tag=f"wef_{k}", name=f"wef_{k}")
                        nc.vector.tensor_copy(out=t[:], in_=w_e[0][k][:])
                        wef.append(t)
                    n1f0 = spool.tile([P, H], BF16, tag="n1f0", name="n1f0")
                    nc.vector.tensor_copy(out=n1f0[:], in_=w_n1[0][0][:])
                    n2f0 = spool.tile([P, H], BF16, tag="n2f0", name="n2f0")
                    nc.vector.tensor_copy(out=n2f0[:], in_=w_n2[0][0][:])
                    bef, b2f = be_col[0], b2_col[0]
                    b1bc = None
                else:
                    wef = []
                    for k in range(3):
                        t = spool.tile([P, H], BF16, tag=f"wef_{k}", name=f"wef_{k}")
                        nc.vector.tensor_scalar(
                            out=t[:], in0=w_e[i][k][:],
                            scalar1=(s_h if k < 2 else s_e)[:, 0:1],
                            scalar2=None, op0=AluOp.mult,
                        )
                        wef.append(t)
                    n1f0 = spool.tile([P, H], BF16, tag="n1f0", name="n1f0")
                    nc.vector.tensor_scalar(
                        out=n1f0[:], in0=w_n1[i][0][:], scalar1=s_h[:, 0:1],
                        scalar2=None, op0=AluOp.mult,
                    )
                    n2f0 = spool.tile([P, H], BF16, tag="n2f0", name="n2f0")
                    nc.vector.tensor_scalar(
                        out=n2f0[:], in0=w_n2[i][0][:], scalar1=s_h[:, 0:1],
                        scalar2=None, op0=AluOp.mult,
                    )
                    psb = ps_m.tile([P, P], F32, tag="psm", name="psb")
                    nc.tensor.matmul(out=psb[:, 0:1], lhsT=w_e[i][0][:],
                                     rhs=t_h[:, 0:1], start=True, stop=False)
                    nc.tensor.matmul(out=psb[:, 0:1], lhsT=w_e[i][1][:],
                                     rhs=t_h[:, 0:1], start=False, stop=False)
                    nc.tensor.matmul(out=psb[:, 0:1], lhsT=w_e[i][2][:],
                                     rhs=t_e[:, 0:1], start=False, stop=True)
                    bef = spool.tile([H, 1], F32, tag="bef", name="bef")
                    nc.vector.tensor_tensor(
                        out=bef[:], in0=psb[:, 0:1], in1=be_col[i][:], op=AluOp.add
                    )
                    psb1 = ps_m.tile([P, P], F32, tag="psm", name="psb1")
                    nc.tensor.matmul(out=psb1[:, 0:1], lhsT=w_n1[i][0][:],
                                     rhs=t_h[:, 0:1], start=True, stop=True)
                    b1f = spool.tile([H, 1], F32, tag="b1f", name="b1f")
                    nc.vector.tensor_tensor(
                        out=b1f[:], in0=psb1[:, 0:1], in1=b1_col[i][:], op=AluOp.add
                    )
                    psb2 = ps_m.tile([P, P], F32, tag="psm", name="psb2")
                    nc.tensor.matmul(out=psb2[:, 0:1], lhsT=w_n2[i][0][:],
                                     rhs=t_h[:, 0:1], start=True, stop=True)
                    b2f = spool.tile([H, 1], F32, tag="b2f", name="b2f")
                    nc.vector.tensor_tensor(
                        out=b2f[:], in0=psb2[:, 0:1], in1=b2_col[i][:], op=AluOp.add
                    )

                    psr = ps_m.tile([P, P], F32, tag="psm", name="psr")
                    nc.tensor.transpose(
                        out=psr[0:1, 0:P], in_=b1f[:, 0:1], identity=ident_f[:]
                    )
                    b1rw = spool.tile([1, P], F32, tag="b1_row", name="b1_row")
                    copy_dve(b1rw[:], psr[0:1, 0:P])
                    psbb = ps_m.tile([P, P], F32, tag="psm", name="psbb")
                    nc.tensor.matmul(
                        out=psbb[:], lhsT=ones_row[:], rhs=b1rw[:],
                        start=True, stop=True,
                    )
                    b1bc = spool.tile([P, P], F32, tag="b1_bc", name="b1_bc")
                    copy_dve(b1bc[:], psbb[:])

                # pad-slot e value for this layer (mirrors the stored bf16 chain)
                pspad = ps_m.tile([P, P], F32, tag="psm", name="pspad")
                nc.tensor.matmul(
                    out=pspad[:, 0:1], lhsT=wef[2][:], rhs=epad_bf[:, 0:1],
                    start=True, stop=True,
                )
                epad_f = spool.tile([P, 1], F32, tag="epad_f", name="epad_f")
                nc.scalar.activation(
                    out=epad_f[:], in_=pspad[:, 0:1], func=Act.Relu, bias=bef[:, 0:1]
                )
                epad_bf = spool.tile(
                    [P, 1], BF16, tag=f"epad_bf_{'b' if i % 2 == 0 else 'a'}",
                    name="epad_bf2",
                )
                nc.vector.tensor_copy(out=epad_bf[:], in_=epad_f[:])
                epx = spool.tile([P, 1], F32, tag="epx", name="epx")
                nc.vector.tensor_copy(out=epx[:], in_=epad_bf[:])

                # ---- hw2 = h_win @ wef1 for all windows (feeds pse) ----
                hw2a = spool.tile([P, NW * P], BF16, tag="hw2a", name="hw2a")
                for s0 in range(0, NPC, 512):
                    s1 = min(s0 + 512, NPC)
                    pshw = ps_e.tile([P, 1024], F32, tag="pse_w",
                                     name="pshw")
                    for ww in range(s0 // P, (s1 + P - 1) // P):
                        nc.tensor.matmul(
                            out=pshw[:, ww * P - s0 : (ww + 1) * P - s0],
                            lhsT=hbf[i % 2][:, ww * P : (ww + 1) * P],
                            rhs=wef[1][:], start=True, stop=True,
                        )
                    nc.vector.tensor_copy(
                        out=hw2a[:, s0:s1], in_=pshw[:, : s1 - s0]
                    )

                # ---- pad-node h value for the NEXT h (mirrors psf1) ----
                psph = ps_m.tile([P, P], F32, tag="psm", name="psph")
                nc.tensor.matmul(
                    out=psph[:, 0:1], lhsT=n2f0[:], rhs=hpad[:, 0:1],
                    start=True, stop=True,
                )
                hpad = spool.tile(
                    [P, 1], BF16, tag=f"hpad_{'b' if i % 2 == 0 else 'a'}",
                    name="hpad2",
                )
                nc.scalar.activation(
                    out=hpad[:], in_=psph[:, 0:1], func=Act.Relu, bias=b2f[:, 0:1]
                )
                hpad_f = spool.tile([P, 1], F32, tag="hpad_f", name="hpad_f")
                nc.vector.tensor_copy(out=hpad_f[:], in_=hpad[:])

                # ---- stats accumulators ----
                se_cols = spool.tile([P, 2 * NW], F32, tag="se_cols",
                                     name="se_cols")
                se2_cols = spool.tile([P, NW], F32, tag="se2_cols", name="se2_cols")

                # ---- window sweep ----
                glo_t = glo_pref
                glo_pref = {}
                for g in range(NG):
                    w0 = g * GW
                    gsz = min(GW, NW - w0)
                    glo = glo_t.pop(g)
                    ghi = gpool.tile([P, 1, NIRH], BF16, tag="ghi", name="ghi")
                    nc.gpsimd.dma_gather(
                        out_ap=ghi[:], in_ap=htabB[i][:, :],
                        idxs_ap=ihi_sb[:, g * SWH : (g + 1) * SWH],
                        num_idxs=NIRH, num_idxs_reg=NIRH, elem_size=H,
                        transpose=True, single_packet=False,
                    )
                    if g + 3 < NG and g > 0:
                        glo_t[g + 3] = issue_glo_for(i, g + 3)
                    elif g == 0:
                        for gn in (2, 3):
                            if gn < NG:
                                glo_t[gn] = issue_glo_for(i, gn)
                    if not last and g == NG - 1:
                        nc.gpsimd.collective_compute(
                            "AllGather", AluOp.bypass, replica_groups=rg,
                            ins=[hsegA[i + 1][:, :]], outs=[htabA[i + 1][:, :]],
                        )
                        for gg in range(min(2, NG)):
                            glo_pref[gg] = issue_glo_for(i + 1, gg)
                    t1w_sb = stpool.tile([P, GW * WSL], BF16, tag="t1w_sb",
                                        name="t1w_sb")
                    nc.sync.dma_start(
                        out=t1w_sb[:, : gsz * WSL],
                        in_=t1w_p[:, w0 * WSL : (w0 + gsz) * WSL],
                    )
                    oht_sb = stpool.tile([P, GW * WSL], BF16, tag="oht_sb",
                                        name="oht_sb")
                    nc.sync.dma_start(
                        out=oht_sb[:, : gsz * WSL],
                        in_=oht_p[:, w0 * WSL : (w0 + gsz) * WSL],
                    )
                    et_g = stpool.tile([P, GW * WSL], BF16, tag="et_g",
                                      name="et_g")
                    nc.sync.dma_start(
                        out=et_g[:, : gsz * WSL],
                        in_=eT_cur[:, w0 * WSL : (w0 + gsz) * WSL],
                    )
                    for j in range(gsz):
                        w = w0 + j
                        ws, es = w * P, w * WSL
                        js = j * WSL
                        jl, jh = j * LOW, j * HIW
                        hlo = glo[:, 0, jl : jl + LOW]
                        hhi = ghi[:, 0, jh : jh + HIW]

                        pse = ps_e.tile([P, 1024], F32, tag="pse_w",
                                        name="pse")
                        enT = wpool.tile([P, WSL], BF16, tag="enT", name="enT")
                        regions = ((0, 0, LOW, hlo), (PSE_HI, LOW, WSL, hhi))
                        for ri, (ps0, e0, e1, hsrc) in enumerate(regions):
                            nc.tensor.matmul(
                                out=pse[:, ps0 : ps0 + e1 - e0], lhsT=wef[0][:],
                                rhs=hsrc, start=True, stop=False,
                            )
                            nc.tensor.matmul(
                                out=pse[:, ps0 : ps0 + e1 - e0], lhsT=wef[2][:],
                                rhs=et_g[:, js + e0 : js + e1],
                                start=False, stop=False,
                            )
                            nc.tensor.matmul(
                                out=pse[:, ps0 : ps0 + e1 - e0],
                                lhsT=hw2a[:, ws : ws + P],
                                rhs=oht_sb[:, js + e0 : js + e1],
                                start=False, stop=True,
                            )
                            nc.scalar.activation(
                                out=enT[:, e0:e1], in_=pse[:, ps0 : ps0 + e1 - e0],
                                func=Act.Relu, bias=bef[:, 0:1],
                                accum_out=se_cols[:, 2 * w + ri : 2 * w + ri + 1],
                            )
                        if not last:
                            dump_e = wpool.tile([P, WSL], BF16, tag="dump_e",
                                                name="dump_e")
                            nc.scalar.activation(
                                out=dump_e[:], in_=enT[:], func=Act.Square,
                                accum_out=se2_cols[:, w : w + 1],
                            )
                            nc.sync.dma_start(out=eT_nxt[:, es : es + WSL],
                                              in_=enT[:])

                        psa = ps_agg.tile([P, P], F32, tag="psa", name="psa")
                        for ck in range(CPW):
                            cs = ck * P
                            if ck < cfg.LCH:
                                hpart = glo[:, 0, jl + cs : jl + cs + P]
                            else:
                                hpart = ghi[:, 0, jh + cs - LOW : jh + cs - LOW + P]
                            psm = ps_m.tile([P, P], F32, tag="psm", name="psm")
                            if b1bc is not None:
                                nc.vector.tensor_copy(out=psm[:], in_=b1bc[:])
                            nc.tensor.matmul(
                                out=psm[:], lhsT=hpart, rhs=n1f0[:],
                                start=(b1bc is None), stop=False,
                                skip_group_check=(b1bc is not None),
                            )
                            nc.tensor.matmul(
                                out=psm[:], lhsT=enT[:, cs : cs + P],
                                rhs=n1b1[i][:], start=False, stop=True,
                                skip_group_check=(b1bc is not None),
                            )
                            m_sb = cpool.tile([P, P], BF16, tag="m_sb", name="m_sb")
                            nc.vector.tensor_scalar(
                                out=m_sb[:], in0=psm[:], scalar1=0.0,
                                scalar2=None, op0=AluOp.max,
                            )
                            nc.tensor.matmul(
                                out=psa[:], lhsT=m_sb[:],
                                rhs=t1w_sb[:, js + cs : js + cs + P],
                                start=(ck == 0), stop=(ck == CPW - 1),
                            )
                        aggT = cpool.tile([P, P], BF16, tag="aggT", name="aggT")
                        copy_dve(aggT[:], psa[:])

                        psf1 = ps_m.tile([P, P], F32, tag="psm", name="psf1")
                        nc.tensor.matmul(
                            out=psf1[:], lhsT=n2f0[:], rhs=h_cur[:, ws : ws + P],
                            start=True, stop=False,
                        )
                        nc.tensor.matmul(
                            out=psf1[:], lhsT=n2b1[i][:], rhs=aggT[:],
                            start=False, stop=True,
                        )
                        nc.scalar.activation(
                            out=h_nxt[:, ws : ws + P], in_=psf1[:],
                            func=Act.Relu, bias=b2f[:, 0:1],
                        )
                        if not last:
                            pst = ps_m.tile([P, P], BF16, tag="psm", name="pst")
                            nc.tensor.transpose(
                                out=pst[:], in_=h_nxt[:, ws : ws + P],
                                identity=ident_bf[:],
                            )
                            hwin = cpool.tile([P, P], BF16, tag="hwin",
                                              name="hwin")
                            nc.scalar.activation(out=hwin[:], in_=pst[:],
                                                 func=Act.Copy)
                            if w < cfg.WA:
                                nc.sync.dma_start(
                                    out=hsegA[i + 1][ws : ws + P, :],
                                    in_=hwin[:],
                                )
                            else:
                                wb = (w - cfg.WA) * P
                                nc.sync.dma_start(
                                    out=hsegB[i + 1][wb : wb + P, :],
                                    in_=hwin[:],
                                )

                # ---- end of layer: stats AllReduce ----
                ar_sb = spool.tile([P, 4], F32, tag="ar_sb", name="ar_sb")
                nc.vector.tensor_reduce(
                    out=ar_sb[:, 0:1], in_=se_cols[:], axis=mybir.AxisListType.X,
                    op=AluOp.add,
                )
                ecor = spool.tile([P, 1], F32, tag="ecor", name="ecor")
                nc.vector.tensor_tensor(
                    out=ecor[:], in0=epad_f[:], in1=npads_sb[:], op=AluOp.mult
                )
                nc.vector.tensor_tensor(
                    out=ar_sb[:, 0:1], in0=ar_sb[:, 0:1], in1=ecor[:],
                    op=AluOp.subtract,
                )
                if not last:
                    nc.vector.tensor_reduce(
                        out=ar_sb[:, 1:2], in_=se2_cols[:],
                        axis=mybir.AxisListType.X, op=AluOp.add,
                    )
                    esq = spool.tile([P, 1], F32, tag="esq", name="esq")
                    nc.vector.tensor_tensor(
                        out=esq[:], in0=epx[:], in1=epx[:], op=AluOp.mult
                    )
                    ecor2 = spool.tile([P, 1], F32, tag="ecor2", name="ecor2")
                    nc.vector.tensor_tensor(
                        out=ecor2[:], in0=esq[:], in1=npads_sb[:], op=AluOp.mult
                    )
                    nc.vector.tensor_tensor(
                        out=ar_sb[:, 1:2], in0=ar_sb[:, 1:2], in1=ecor2[:],
                        op=AluOp.subtract,
                    )
                else:
                    nc.vector.memset(ar_sb[:, 1:2], 0.0)
                nc.vector.tensor_reduce(
                    out=ar_sb[:, 2:3], in_=h_nxt[:, :], axis=mybir.AxisListType.X,
                    op=AluOp.add,
                )
                hcor = spool.tile([P, 1], F32, tag="hcor", name="hcor")
                nc.vector.tensor_tensor(
                    out=hcor[:], in0=hpad_f[:], in1=npadn_sb[:], op=AluOp.mult
                )
                nc.vector.tensor_tensor(
                    out=ar_sb[:, 2:3], in0=ar_sb[:, 2:3], in1=hcor[:],
                    op=AluOp.subtract,
                )
                if not last:
                    sq_cols = spool.tile([P, (NPC + 511) // 512], F32,
                                         tag="sq_cols", name="sq_cols")
                    for ci, s0 in enumerate(range(0, NPC, 512)):
                        s1 = min(s0 + 512, NPC)
                        dumph = wpool.tile([P, 512], BF16, tag="dumph",
                                           name="dumph")
                        nc.scalar.activation(
                            out=dumph[:, : s1 - s0], in_=h_nxt[:, s0:s1],
                            func=Act.Square, accum_out=sq_cols[:, ci : ci + 1],
                        )
                    nc.vector.tensor_reduce(
                        out=ar_sb[:, 3:4], in_=sq_cols[:],
                        axis=mybir.AxisListType.X, op=AluOp.add,
                    )
                    hsq = spool.tile([P, 1], F32, tag="hsq", name="hsq")
                    nc.vector.tensor_tensor(
                        out=hsq[:], in0=hpad_f[:], in1=hpad_f[:], op=AluOp.mult
                    )
                    hcor2 = spool.tile([P, 1], F32, tag="hcor2", name="hcor2")
                    nc.vector.tensor_tensor(
                        out=hcor2[:], in0=hsq[:], in1=npadn_sb[:], op=AluOp.mult
                    )
                    nc.vector.tensor_tensor(
                        out=ar_sb[:, 3:4], in0=ar_sb[:, 3:4], in1=hcor2[:],
                        op=AluOp.subtract,
                    )
                else:
                    nc.vector.memset(ar_sb[:, 3:4], 0.0)
                nc.sync.dma_start(out=ar_in[i][:, :], in_=ar_sb[:])
                nc.gpsimd.collective_compute(
                    "AllReduce", AluOp.add, replica_groups=rg,
                    ins=[ar_in[i][:, :]], outs=[ar_out[i][:, :]],
                )
                if not last:
                    nc.gpsimd.collective_compute(
                        "AllGather", AluOp.bypass, replica_groups=rg,
                        ins=[hsegB[i + 1][:, :]], outs=[htabB[i + 1][:, :]],
                    )
                arr = spool.tile([P, 4], F32, tag="arr", name="arr")
                nc.sync.dma_start(out=arr[:], in_=ar_out[i][:, :])

                if not last:
                    def bn_consts(sum_c, sq_c, inv_cnt, g_t, b_t, tag):
                        mean = spool.tile([P, 1], F32, tag=f"mean_{tag}",
                                          name=f"mean_{tag}")
                        nc.vector.tensor_scalar(
                            out=mean[:], in0=sum_c, scalar1=inv_cnt, scalar2=None,
                            op0=AluOp.mult,
                        )
                        var = spool.tile([P, 1], F32, tag=f"var_{tag}",
                                         name=f"var_{tag}")
                        nc.vector.tensor_scalar(
                            out=var[:], in0=sq_c, scalar1=inv_cnt, scalar2=None,
                            op0=AluOp.mult,
                        )
                        m2 = spool.tile([P, 1], F32, tag=f"m2_{tag}",
                                        name=f"m2_{tag}")
                        nc.vector.tensor_tensor(
                            out=m2[:], in0=mean[:], in1=mean[:], op=AluOp.mult
                        )
                        nc.vector.tensor_tensor(
                            out=var[:], in0=var[:], in1=m2[:], op=AluOp.subtract
                        )
                        sd = spool.tile([P, 1], F32, tag=f"sd_{tag}",
                                        name=f"sd_{tag}")
                        nc.scalar.activation(
                            out=sd[:], in_=var[:], func=Act.Sqrt,
                            bias=eps_sb[:, 0:1],
                        )
                        rs = spool.tile([P, 1], F32, tag=f"rs_{tag}",
                                        name=f"rs_{tag}")
                        nc.vector.reciprocal(out=rs[:], in_=sd[:])
                        s = spool.tile([P, 1], F32, tag=f"s_{tag}",
                                       name=f"s_{tag}")
                        nc.vector.tensor_tensor(
                            out=s[:], in0=rs[:], in1=g_t[:], op=AluOp.mult
                        )
                        ms = spool.tile([P, 1], F32, tag=f"ms_{tag}",
                                        name=f"ms_{tag}")
                        nc.vector.tensor_tensor(
                            out=ms[:], in0=mean[:], in1=s[:], op=AluOp.mult
                        )
                        t = spool.tile([P, 1], F32, tag=f"t_{tag}",
                                       name=f"t_{tag}")
                        nc.vector.tensor_tensor(
                            out=t[:], in0=b_t[:], in1=ms[:], op=AluOp.subtract
                        )
                        return s, t

                    s_e, t_e = bn_consts(arr[:, 0:1], arr[:, 1:2], inv_e, beg,
                                         beb, "e")
                    s_h, t_h = bn_consts(arr[:, 2:3], arr[:, 3:4], inv_n, bng,
                                         bnb, "h")
                else:
                    roh = spool.tile([P, 1], F32, tag="roh", name="roh")
                    nc.vector.tensor_scalar(
                        out=roh[:], in0=arr[:, 2:3], scalar1=inv_n, scalar2=None,
                        op0=AluOp.mult,
                    )
                    roe = spool.tile([P, 1], F32, tag="roe", name="roe")
                    nc.vector.tensor_scalar(
                        out=roe[:], in0=arr[:, 0:1], scalar1=inv_e, scalar2=None,
                        op0=AluOp.mult,
                    )
                    pso = ps_m.tile([P, P], F32, tag="psm", name="pso")
                    nc.tensor.matmul(
                        out=pso[0:1, 0:1], lhsT=roh[:, 0:1], rhs=regw_h[:, 0:1],
                        start=True, stop=False,
                    )
                    nc.tensor.matmul(
                        out=pso[0:1, 0:1], lhsT=roe[:, 0:1], rhs=regw_e[:, 0:1],
                        start=False, stop=True,
                    )
                    out_sb = spool.tile([1, 1], F32, tag="out_sb", name="out_sb")
                    nc.vector.tensor_tensor(
                        out=out_sb[:], in0=pso[0:1, 0:1], in1=regb_sb[:],
                        op=AluOp.add,
                    )
                    nc.sync.dma_start(out=out_p[:, :], in_=out_sb[:])

    nc.compile()
    return nc


def kernel(**inputs) -> np.ndarray:
    cfg = Cfg()
    maps = prep(cfg, inputs["x"], inputs["edge_index"], inputs["edge_attr"])
    add_weight_params(cfg, maps, inputs)
    nc = build(cfg)
    from concourse.bass_utils import run_bass_kernel_spmd

    res = run_bass_kernel_spmd(nc, maps, list(range(cfg.NC)))
    return np.asarray(res.results[0]["out"], np.float32)

